# revision 13
# baseline (speedup 1.0000x reference)
"""MoE dense all-experts (GPT-OSS Experts forward) on 8 Trainium2 NeuronCores.

Expert-parallel sharding: core e holds expert e's weights and computes its
weighted contribution

    partial_e[t, h] = w[t, e] * ((up + 1) * silu(1.702 * gate) @ down_e.T + db_e)

with [gate | up] = hs @ gup_e + bias (the host de-interleaves gup's even/odd
columns so gate/up become contiguous halves). Each core writes its full
[T, H] partial to DRAM as it is produced and the host sums the 8 partials in
fp32 — there are no on-device collectives, so the cores run fully decoupled
and the kernel ends right after the last tile's store.

Matmuls run in bf16 (both operands, fp32 PSUM accumulation). The fp32r
version drew enough power that the PE was firmware-throttled to K=13/16
(~1.95 GHz) for the whole kernel; bf16 halves the multiplier energy and the
SBUF/DMA traffic, letting the PE hold its full 2.4 GHz clock — measured
matmul issue pace matches the 2.4 GHz model and PE busy time sits at the
N-cycles-per-matmul instruction floor. End-to-end relative error is ~3.6e-3
(gate is 2e-2).

All device tensors are host-pre-transposed into the exact SBUF layouts
(partition-major, fully contiguous) so every DMA moves multi-KB runs per
partition. The gate weights are laid out j-major and streamed strip-by-strip
on the Sync queue while the chunk-0 tokens stream in small pieces on the
Scalar queue — the two queues' ~0.6us per-DMA issue costs overlap, so the
first matmul chain starts as soon as one strip plus one token slice land.
Stage 1 computes [f, t] tiles (gate pass feeding the ScalarE Silu LUT, then
up pass fused with the silu output via scalar_tensor_tensor into bf16
act[i, t]); stage 2 computes out[t, h] with act as the stationary operand.
The down-bias + routing-weight epilogue runs on the VectorE:
out = (psum * w[t]) + w[t]*db[h], with the rank-1 w*db tile built from a
partition-broadcast copy of db. The very last output tile is computed as two
N=256 chains so its epilogue + store tail is half as long.
"""
import sys
if '/opt/trn_rl_repo' not in sys.path:
    sys.path.insert(0, '/opt/trn_rl_repo')
import numpy as np
import ml_dtypes

E, H, I, T = 8, 1024, 1024, 4096
N_CORES = 8
NCH = 8
TC = 512               # tokens per chunk (T == NCH * TC)
KC = H // 128          # contraction chunks (H == I == 1024)
NJ = I // 128          # gate/up row tiles
NTT = TC // 128

_CACHE = {}


def _build():
    import concourse.bacc as bacc
    import concourse.tile as tile
    import concourse.mybir as mybir
    f32 = mybir.dt.float32
    bf16 = mybir.dt.bfloat16
    AF = mybir.ActivationFunctionType
    ALU = mybir.AluOpType

    nc = bacc.Bacc("TRN2", target_bir_lowering=False, debug=False,
                   enable_asserts=False, num_devices=N_CORES)
    # all pre-transposed to SBUF layout on the host (see _make_in_maps)
    hsf = nc.dram_tensor("hsf", [128, NCH * KC * TC], bf16, kind="ExternalInput").ap()
    gupg = nc.dram_tensor("gupg", [128, NJ * KC * 128], bf16, kind="ExternalInput").ap()
    gupu = nc.dram_tensor("gupu", [128, KC * I], bf16, kind="ExternalInput").ap()
    dwTf = nc.dram_tensor("dwTf", [128, KC * H], bf16, kind="ExternalInput").ap()
    gb = nc.dram_tensor("gb", [128, NJ], f32, kind="ExternalInput").ap()
    ub = nc.dram_tensor("ub", [128, NJ], f32, kind="ExternalInput").ap()
    db = nc.dram_tensor("db", [1, H], f32, kind="ExternalInput").ap()
    wt = nc.dram_tensor("wt", [128, T // 128], f32, kind="ExternalInput").ap()
    opart = nc.dram_tensor("opart", [T, H], f32, kind="ExternalOutput").ap()

    SJ = KC * 128        # columns per j-strip of the gate half

    with tile.TileContext(nc) as tc_:
        with tc_.tile_pool(name="wpool", bufs=1) as wpool, \
             tc_.tile_pool(name="hpool", bufs=2) as hpool, \
             tc_.tile_pool(name="apool", bufs=2) as apool, \
             tc_.tile_pool(name="spool", bufs=8) as spool, \
             tc_.tile_pool(name="opool", bufs=3) as opool, \
             tc_.tile_pool(name="bpool", bufs=4) as bpool, \
             tc_.tile_pool(name="ps1", bufs=2, space="PSUM") as ps1, \
             tc_.tile_pool(name="ps2", bufs=3, space="PSUM") as ps2:

            gupg_r = wpool.tile([128, NJ * SJ], bf16)
            gupu_r = wpool.tile([128, KC * I], bf16)
            dwT_r = wpool.tile([128, KC * H], bf16)
            gb_r = wpool.tile([128, NJ], f32)
            ub_r = wpool.tile([128, NJ], f32)
            db_f = wpool.tile([1, H], f32)
            db_bc = wpool.tile([128, H], f32)
            w_r = wpool.tile([128, T // 128], f32)

            # Weight stream on the Sync queue, token stream on the Scalar
            # queue. Chunk-0 tokens go in graduated pieces (1,1,2,2,2 kc) so
            # the first gate chain's k-accumulation is paced without paying
            # eight serial issue costs.
            hs0 = hpool.tile([128, KC * TC], bf16, tag="hs")
            for a, b in ((0, 1), (1, 2), (2, 4), (4, 6), (6, 8)):
                nc.scalar.dma_start(hs0[:, a*TC:b*TC], hsf[:, a*TC:b*TC])
            nc.scalar.dma_start(gb_r[:], gb[:])
            nc.scalar.dma_start(ub_r[:], ub[:])
            hs1 = hpool.tile([128, KC * TC], bf16, tag="hs")
            nc.scalar.dma_start(hs1[:], hsf[:, KC * TC:2 * KC * TC])

            nc.sync.dma_start(gupg_r[:, 0:SJ], gupg[:, 0:SJ])
            nc.sync.dma_start(gupg_r[:, SJ:2 * SJ], gupg[:, SJ:2 * SJ])
            for j2 in range(1, NJ // 2):
                nc.sync.dma_start(gupg_r[:, 2*j2*SJ:2*(j2+1)*SJ],
                                  gupg[:, 2*j2*SJ:2*(j2+1)*SJ])
            nc.sync.dma_start(gupu_r[:], gupu[:])
            nc.sync.dma_start(db_f[:], db[:])
            nc.sync.dma_start(w_r[:], wt[:])
            nc.sync.dma_start(dwT_r[:], dwTf[:])
            nc.gpsimd.partition_broadcast(db_bc[:], db_f[:])

            for c in range(NCH):
                if c == 0:
                    hs_r = hs0
                elif c == 1:
                    hs_r = hs1
                else:
                    hs_r = hpool.tile([128, KC * TC], bf16, tag="hs")
                    nc.scalar.dma_start(hs_r[:], hsf[:, c*KC*TC:(c+1)*KC*TC])

                act_r = apool.tile([128, NJ * TC], bf16, tag="act")
                s2s = []
                for j in range(NJ):     # gate pass
                    pg = ps1.tile([128, TC], f32, tag="pg")
                    for kc in range(KC):
                        nc.tensor.matmul(pg[:], gupg_r[:, j*SJ + kc*128 : j*SJ + (kc+1)*128],
                                         hs_r[:, kc*TC:(kc+1)*TC],
                                         start=(kc == 0), stop=(kc == KC - 1))
                    s2 = spool.tile([128, TC], f32, tag="s2")
                    nc.scalar.activation(s2[:], pg[:], AF.Silu,
                                         bias=gb_r[:, j:j+1], scale=1.702)
                    s2s.append(s2)
                for j in range(NJ):     # up pass: act = (up + ub + 1) * silu_out
                    pu = ps1.tile([128, TC], f32, tag="pu")
                    for kc in range(KC):
                        nc.tensor.matmul(pu[:], gupu_r[:, kc*I + j*128 : kc*I + (j+1)*128],
                                         hs_r[:, kc*TC:(kc+1)*TC],
                                         start=(kc == 0), stop=(kc == KC - 1))
                    nc.vector.scalar_tensor_tensor(act_r[:, j*TC:(j+1)*TC], pu[:],
                                                   ub_r[:, j:j+1], s2s[j][:],
                                                   op0=ALU.add, op1=ALU.mult)

                for tt in range(NTT):
                    gt = c * NTT + tt
                    wcol = w_r[:, gt:gt+1]
                    ot = opool.tile([128, H], f32, tag="ot")
                    for hh in range(H // 512):
                        dbw = bpool.tile([128, 512], f32, tag="dbw")
                        nc.vector.tensor_scalar_mul(dbw[:], db_bc[:, hh*512:(hh+1)*512], wcol)
                        # the kernel's very last tile runs as two N=256
                        # chains so the closing stt + store tail is shorter
                        last_tile = (c == NCH - 1 and tt == NTT - 1 and hh == 1)
                        splits = ((0, 256), (256, 512)) if last_tile else ((0, 512),)
                        for lo, hi in splits:
                            p2 = ps2.tile([128, 512], f32, tag="p2")
                            for ic in range(KC):
                                nc.tensor.matmul(p2[:, 0:hi-lo],
                                                 act_r[:, ic*TC + tt*128 : ic*TC + (tt+1)*128],
                                                 dwT_r[:, ic*H + hh*512 + lo : ic*H + hh*512 + hi],
                                                 start=(ic == 0), stop=(ic == KC - 1))
                            nc.vector.scalar_tensor_tensor(
                                ot[:, hh*512 + lo : hh*512 + hi], p2[:, 0:hi-lo], wcol,
                                dbw[:, lo:hi], op0=ALU.mult, op1=ALU.add)
                            nc.sync.dma_start(
                                opart[gt*128:(gt+1)*128, hh*512 + lo : hh*512 + hi],
                                ot[:, hh*512 + lo : hh*512 + hi])
    nc.compile()
    return nc


def _get_nc():
    if 'nc' not in _CACHE:
        _CACHE['nc'] = _build()
    return _CACHE['nc']


def _make_in_maps(hidden_states, routing_weights, gate_up_proj, gate_up_proj_bias,
                  down_proj, down_proj_bias):
    bf16 = ml_dtypes.bfloat16
    hs = np.asarray(hidden_states, dtype=np.float32)
    rw = np.asarray(routing_weights, dtype=np.float32)
    gupw = np.asarray(gate_up_proj, dtype=np.float32)
    gupb = np.asarray(gate_up_proj_bias, dtype=np.float32)
    dw = np.asarray(down_proj, dtype=np.float32)
    dbias = np.asarray(down_proj_bias, dtype=np.float32)
    # hsT[kc*128+p, c*TC+t] -> hsf[p, c*(KC*TC) + kc*TC + t]  (chunk-major)
    hsT = hs.T.astype(bf16)
    hsf = np.ascontiguousarray(
        hsT.reshape(KC, 128, NCH, TC).transpose(1, 2, 0, 3).reshape(128, NCH * KC * TC))
    in_maps = []
    for e in range(N_CORES):
        g = gupw[e]
        g_gate = (g[:, 0::2]).astype(bf16)   # [H, I]
        g_up = (g[:, 1::2]).astype(bf16)     # [H, I]
        # g_gate[kc*128+p, j*128+c] -> gupg[p, j*(KC*128) + kc*128 + c]  (j-major)
        gupg = np.ascontiguousarray(
            g_gate.reshape(KC, 128, NJ, 128).transpose(1, 2, 0, 3).reshape(128, NJ * KC * 128))
        # g_up[kc*128+p, col] -> gupu[p, kc*I + col]  (kc-major)
        gupu = np.ascontiguousarray(
            g_up.reshape(KC, 128, I).transpose(1, 0, 2).reshape(128, KC * I))
        # dwT[ic*128+p, h] -> dwTf[p, ic*H + h]; silu's 1.702 scale folded in
        dwT = (dw[e].T / np.float32(1.702)).astype(bf16)
        dwTf = np.ascontiguousarray(
            dwT.reshape(KC, 128, H).transpose(1, 0, 2).reshape(128, KC * H))
        in_maps.append({
            "hsf": hsf,
            "gupg": gupg,
            "gupu": gupu,
            "dwTf": dwTf,
            # silu(1.702*(x + b)) = silu(1.702*x + 1.702*b)
            "gb": np.ascontiguousarray((1.702 * gupb[e, 0::2]).reshape(NJ, 128).T),
            "ub": np.ascontiguousarray((gupb[e, 1::2] + 1.0).reshape(NJ, 128).T),
            "db": np.ascontiguousarray(dbias[e][None, :]),
            "wt": np.ascontiguousarray(rw[:, e].reshape(T // 128, 128).T),
        })
    return in_maps


def _assemble(results):
    out = results[0]["opart"].astype(np.float32, copy=True)
    for r in range(1, N_CORES):
        np.add(out, results[r]["opart"], out=out)
    return out


def kernel(hidden_states, routing_weights, gate_up_proj, gate_up_proj_bias,
           down_proj, down_proj_bias):
    from concourse import bass_utils
    in_maps = _make_in_maps(hidden_states, routing_weights, gate_up_proj,
                            gate_up_proj_bias, down_proj, down_proj_bias)
    nc = _get_nc()
    try:
        res = bass_utils.run_bass_kernel_spmd(nc, in_maps, core_ids=list(range(N_CORES)))
    except Exception:
        # One retry in case a previous process left a core wedged.
        res = bass_utils.run_bass_kernel_spmd(nc, in_maps, core_ids=list(range(N_CORES)))
    return _assemble(res.results)


# revision 14
# speedup vs baseline: 1.1996x; 1.1996x over previous
"""MoE dense all-experts (GPT-OSS Experts forward) on 8 Trainium2 NeuronCores.

Expert-parallel sharding: core e holds expert e's weights and computes its
weighted contribution

    partial_e[t, h] = w[t, e] * ((up + 1) * silu(1.702 * gate) @ down_e.T + db_e)

with [gate | up] = hs @ gup_e + bias (the host de-interleaves gup's even/odd
columns so gate/up become contiguous halves). Each core writes its full
[T, H] partial to DRAM as it is produced and the host sums the 8 partials in
fp32 — there are no on-device collectives, so the cores run fully decoupled
and the kernel ends right after the last tile's store.

Matmuls run in bf16 (both operands, fp32 PSUM accumulation). The fp32r
version drew enough power that the PE was firmware-throttled to K=13/16
(~1.95 GHz) for the whole kernel; bf16 halves the multiplier energy and the
SBUF/DMA traffic, letting the PE hold its full 2.4 GHz clock — measured
matmul issue pace matches the 2.4 GHz model and PE busy time sits at the
N-cycles-per-matmul instruction floor. End-to-end relative error is ~3.6e-3
(gate is 2e-2).

The gate half of gup streams in four j2-strips of [128, kc, 256] (rearranged
straight out of the row-major dram tensor) so the first gate chain only
waits for strip 0 plus the chunk-0 tokens, and later strips stay ahead of
the PE; the up half / down weights are consumed much later and load kc-major.
Stage 1 computes [f, t] tiles (gate pass feeding the ScalarE Silu LUT, then
up pass fused with the silu output via scalar_tensor_tensor into bf16
act[i, t]); stage 2 computes out[t, h] with act as the stationary operand.
The down-bias + routing-weight epilogue runs on the VectorE:
out = (psum * w[t]) + w[t]*db[h], with the rank-1 w*db tile built from a
partition-broadcast copy of db.
"""
import sys
if '/opt/trn_rl_repo' not in sys.path:
    sys.path.insert(0, '/opt/trn_rl_repo')
import numpy as np
import ml_dtypes

E, H, I, T = 8, 1024, 1024, 4096
N_CORES = 8
CHUNKS = [512] * 8
KC = H // 128          # contraction chunks (H == I == 1024)
NJ = I // 128          # gate/up row tiles
TCMAX = max(CHUNKS)

_CACHE = {}


def _build():
    import concourse.bacc as bacc
    import concourse.tile as tile
    import concourse.mybir as mybir
    f32 = mybir.dt.float32
    bf16 = mybir.dt.bfloat16
    AF = mybir.ActivationFunctionType
    ALU = mybir.AluOpType

    nc = bacc.Bacc("TRN2", target_bir_lowering=False, debug=False,
                   enable_asserts=False, num_devices=N_CORES)
    hsT = nc.dram_tensor("hsT", [H, T], bf16, kind="ExternalInput").ap()
    gup = nc.dram_tensor("gup", [H, 2 * I], bf16, kind="ExternalInput").ap()
    gb = nc.dram_tensor("gb", [128, NJ], f32, kind="ExternalInput").ap()
    ub = nc.dram_tensor("ub", [128, NJ], f32, kind="ExternalInput").ap()
    dwT = nc.dram_tensor("dwT", [I, H], bf16, kind="ExternalInput").ap()
    db = nc.dram_tensor("db", [1, H], f32, kind="ExternalInput").ap()
    wt = nc.dram_tensor("wt", [128, T // 128], f32, kind="ExternalInput").ap()
    opart = nc.dram_tensor("opart", [T, H], f32, kind="ExternalOutput").ap()

    with tile.TileContext(nc) as tc_:
        with tc_.tile_pool(name="wpool", bufs=1) as wpool, \
             tc_.tile_pool(name="hpool", bufs=2) as hpool, \
             tc_.tile_pool(name="apool", bufs=2) as apool, \
             tc_.tile_pool(name="spool", bufs=8) as spool, \
             tc_.tile_pool(name="opool", bufs=3) as opool, \
             tc_.tile_pool(name="bpool", bufs=4) as bpool, \
             tc_.tile_pool(name="ps1", bufs=2, space="PSUM") as ps1, \
             tc_.tile_pool(name="ps2", bufs=3, space="PSUM") as ps2:

            gupg_r = wpool.tile([128, NJ * KC * 128], bf16)   # gate, j2-strip-major
            gupu_r = wpool.tile([128, KC * I], bf16)          # up, kc-major
            dwT_r = wpool.tile([128, KC * H], bf16)
            gb_r = wpool.tile([128, NJ], f32)
            ub_r = wpool.tile([128, NJ], f32)
            db_f = wpool.tile([1, H], f32)
            db_bc = wpool.tile([128, H], f32)
            w_r = wpool.tile([128, T // 128], f32)

            # DMA order matches consumption order. The gate half streams in
            # four j2-strips of [128, kc, 256] (rearranged straight out of the
            # row-major dram tensor) so the first gate chain only waits for
            # strip 0 + the chunk-0 tokens, and later strips stay one chain
            # ahead of the PE. The up half / down weights are consumed much
            # later and load kc-major (plain 2KB-per-partition rows).
            def gate_strip(j2):
                nc.sync.dma_start(
                    gupg_r[:, j2*KC*256:(j2+1)*KC*256].rearrange("p (kc c) -> p kc c", c=256),
                    gup[:, j2*256:(j2+1)*256].rearrange("(kc p) c -> p kc c", p=128))
            gate_strip(0)
            hs0 = hpool.tile([128, KC * TCMAX], bf16, tag="hs")
            for kc in range(KC):
                nc.sync.dma_start(hs0[:, kc*TCMAX:kc*TCMAX + CHUNKS[0]],
                                  hsT[kc*128:(kc+1)*128, 0:CHUNKS[0]])
            nc.sync.dma_start(gb_r[:], gb[:])
            nc.sync.dma_start(ub_r[:], ub[:])
            for j2 in range(1, NJ // 2):
                gate_strip(j2)
            for kc in range(KC):
                nc.sync.dma_start(gupu_r[:, kc*I:(kc+1)*I],
                                  gup[kc*128:(kc+1)*128, I:2*I])
            nc.sync.dma_start(db_f[:], db[:])
            nc.sync.dma_start(w_r[:], wt[:])
            hs1 = hpool.tile([128, KC * TCMAX], bf16, tag="hs")
            nc.sync.dma_start(
                hs1[:].rearrange("p (kc t) -> p kc t", t=TCMAX)[:, :, 0:CHUNKS[1]],
                hsT[:, CHUNKS[0]:CHUNKS[0] + CHUNKS[1]].rearrange("(kc p) t -> p kc t", p=128))
            for kc in range(KC):
                nc.sync.dma_start(dwT_r[:, kc*H:(kc+1)*H], dwT[kc*128:(kc+1)*128, :])
            nc.gpsimd.partition_broadcast(db_bc[:], db_f[:])

            t_off = 0
            for c, TC in enumerate(CHUNKS):
                NTT = TC // 128
                if c == 0:
                    hs_r = hs0
                elif c == 1:
                    hs_r = hs1
                else:
                    hs_r = hpool.tile([128, KC * TCMAX], bf16, tag="hs")
                    nc.sync.dma_start(
                        hs_r[:].rearrange("p (kc t) -> p kc t", t=TCMAX)[:, :, 0:TC],
                        hsT[:, t_off:t_off + TC].rearrange("(kc p) t -> p kc t", p=128))

                act_r = apool.tile([128, NJ * TCMAX], bf16, tag="act")
                s2s = []
                for j in range(NJ):     # gate pass
                    pg = ps1.tile([128, TC], f32, tag="pg")
                    base = (j // 2) * KC * 256 + (j % 2) * 128
                    for kc in range(KC):
                        nc.tensor.matmul(pg[:], gupg_r[:, base + kc*256 : base + kc*256 + 128],
                                         hs_r[:, kc*TCMAX:kc*TCMAX + TC],
                                         start=(kc == 0), stop=(kc == KC - 1))
                    s2 = spool.tile([128, TCMAX], f32, tag="s2")
                    nc.scalar.activation(s2[:, :TC], pg[:], AF.Silu,
                                         bias=gb_r[:, j:j+1], scale=1.702)
                    s2s.append(s2)
                for j in range(NJ):     # up pass: act = (up + ub + 1) * silu_out
                    pu = ps1.tile([128, TC], f32, tag="pu")
                    for kc in range(KC):
                        nc.tensor.matmul(pu[:], gupu_r[:, kc*I + j*128 : kc*I + (j+1)*128],
                                         hs_r[:, kc*TCMAX:kc*TCMAX + TC],
                                         start=(kc == 0), stop=(kc == KC - 1))
                    nc.vector.scalar_tensor_tensor(act_r[:, j*TCMAX:j*TCMAX + TC], pu[:],
                                                   ub_r[:, j:j+1], s2s[j][:, :TC],
                                                   op0=ALU.add, op1=ALU.mult)

                for tt in range(NTT):
                    gt = (t_off // 128) + tt
                    wcol = w_r[:, gt:gt+1]
                    ot = opool.tile([128, H], f32, tag="ot")
                    for hh in range(H // 512):
                        dbw = bpool.tile([128, 512], f32, tag="dbw")
                        nc.vector.tensor_scalar_mul(dbw[:], db_bc[:, hh*512:(hh+1)*512], wcol)
                        p2 = ps2.tile([128, 512], f32, tag="p2")
                        for ic in range(KC):
                            nc.tensor.matmul(p2[:], act_r[:, ic*TCMAX + tt*128 : ic*TCMAX + (tt+1)*128],
                                             dwT_r[:, ic*H + hh*512 : ic*H + (hh+1)*512],
                                             start=(ic == 0), stop=(ic == KC - 1))
                        nc.vector.scalar_tensor_tensor(ot[:, hh*512:(hh+1)*512], p2[:], wcol,
                                                       dbw[:], op0=ALU.mult, op1=ALU.add)
                        nc.sync.dma_start(opart[gt*128:(gt+1)*128, hh*512:(hh+1)*512],
                                          ot[:, hh*512:(hh+1)*512])
                t_off += TC
    nc.compile()
    return nc


def _get_nc():
    if 'nc' not in _CACHE:
        _CACHE['nc'] = _build()
    return _CACHE['nc']


def _make_in_maps(hidden_states, routing_weights, gate_up_proj, gate_up_proj_bias,
                  down_proj, down_proj_bias):
    bf16 = ml_dtypes.bfloat16
    hs = np.asarray(hidden_states, dtype=np.float32)
    rw = np.asarray(routing_weights, dtype=np.float32)
    gupw = np.asarray(gate_up_proj, dtype=np.float32)
    gupb = np.asarray(gate_up_proj_bias, dtype=np.float32)
    dw = np.asarray(down_proj, dtype=np.float32)
    dbias = np.asarray(down_proj_bias, dtype=np.float32)
    hsT = np.ascontiguousarray(hs.T.astype(bf16))
    in_maps = []
    for e in range(N_CORES):
        g = gupw[e]
        gup_de = np.concatenate([g[:, 0::2], g[:, 1::2]], axis=1)
        in_maps.append({
            "hsT": hsT,
            "gup": np.ascontiguousarray(gup_de.astype(bf16)),
            # silu(1.702*(x + b)) = silu(1.702*x + 1.702*b); the 1/1.702 glu
            # scale is folded into dwT below.
            "gb": np.ascontiguousarray((1.702 * gupb[e, 0::2]).reshape(NJ, 128).T),
            "ub": np.ascontiguousarray((gupb[e, 1::2] + 1.0).reshape(NJ, 128).T),
            "dwT": np.ascontiguousarray((dw[e].T / np.float32(1.702)).astype(bf16)),
            "db": np.ascontiguousarray(dbias[e][None, :]),
            "wt": np.ascontiguousarray(rw[:, e].reshape(T // 128, 128).T),
        })
    return in_maps


def _assemble(results):
    out = results[0]["opart"].astype(np.float32, copy=True)
    for r in range(1, N_CORES):
        np.add(out, results[r]["opart"], out=out)
    return out


def kernel(hidden_states, routing_weights, gate_up_proj, gate_up_proj_bias,
           down_proj, down_proj_bias):
    from concourse import bass_utils
    in_maps = _make_in_maps(hidden_states, routing_weights, gate_up_proj,
                            gate_up_proj_bias, down_proj, down_proj_bias)
    nc = _get_nc()
    try:
        res = bass_utils.run_bass_kernel_spmd(nc, in_maps, core_ids=list(range(N_CORES)))
    except Exception:
        # One retry in case a previous process left a core wedged.
        res = bass_utils.run_bass_kernel_spmd(nc, in_maps, core_ids=list(range(N_CORES)))
    return _assemble(res.results)
